# revision 10
# baseline (speedup 1.0000x reference)
"""Multi-head attention (B=4, L=2048, D=1024, H=16) on 8 trn2 NeuronCores.

Sharding: core c = 2*b + g handles batch b and head-group g (8 heads = 512 dims).
Each core computes Q/K/V projections for its group, attention for its 8 heads,
and a partial output projection ctx_g @ Wo[g*512:(g+1)*512, :].  The host sums
the two group partials per batch.

v3 (per core):
  storage dtype bf16 end-to-end (x, weights, QT/KT/V, probs, ctx); PSUM fp32
  QT, KT   : (512, 2048) feature-major (4 tiles of (128, L), 2 heads per tile)
  V        : (2048, 520) token-major, 65 cols per head (64 V dims + ones col
             accumulating the softmax denominator during the ctx matmul)
  scores   : head-pair fused in one (128 k, 1024 q) 2-bank PSUM tile;
             ONE exp per key tile (padd folded into the exp bias);
             causal diag masking AFTER exp by one gpsimd affine_select
  norm     : Z row recip on DVE -> 1/Z broadcast into rows 64:128 of the ctx
             PSUM bank via a tiny PE matmul -> fused (ctx*1)*(1/Z) copy-out
             on DVE scalar_tensor_tensor
  schedule : engines run in emission order, so proj(t+1) matmul chains are
             EMITTED interleaved into attn(t)'s head loops to keep PE busy
             during the ACT(exp)-heavy attention phases
  out      : ctxT.T @ Wo chunks; fp32 partials DMAd out, host sums pairs
"""

import sys

if "/opt/trn_rl_repo" not in sys.path:
    sys.path.insert(0, "/opt/trn_rl_repo")

import numpy as np
from ml_dtypes import bfloat16

B, L, D, H = 4, 2048, 1024, 16
G = 2                # head-groups == cores per batch
DG = D // G          # 512 dims per group
HG = H // G          # 8 heads per group
DH = D // H          # 64
NCORES = B * G
NT = L // 512        # query 512-blocks
NKT = L // 128       # key 128-tiles
ND = D // 128        # contraction chunks over input dim
NJ = DG // 128       # dcol tiles per group (2 heads each)

MM_DTYPE = "bfloat16"

_cache = {}


def _build(mm_dtype_name, causal):
    import concourse.bass as bass
    import concourse.tile as tile
    from concourse import bacc, mybir

    f32 = mybir.dt.float32
    AF = mybir.ActivationFunctionType
    ALU = mybir.AluOpType
    mdt = getattr(mybir.dt, mm_dtype_name)

    nc = bacc.Bacc("TRN2")

    xqt = nc.dram_tensor("xqt", [D, L], mdt, kind="ExternalInput")
    xkt = nc.dram_tensor("xkt", [D, L], mdt, kind="ExternalInput")
    xvt = nc.dram_tensor("xvt", [D, L], mdt, kind="ExternalInput")
    wq_d = nc.dram_tensor("wq", [D, DG], mdt, kind="ExternalInput")
    wk_d = nc.dram_tensor("wk", [D, DG], mdt, kind="ExternalInput")
    wv_d = nc.dram_tensor("wv", [D, DG], mdt, kind="ExternalInput")
    bq_d = nc.dram_tensor("bq", [128, NJ], f32, kind="ExternalInput")
    bk_d = nc.dram_tensor("bk", [128, NJ], f32, kind="ExternalInput")
    bv_d = nc.dram_tensor("bv", [DG], f32, kind="ExternalInput")
    wo_d = nc.dram_tensor("wo", [DG, D], mdt, kind="ExternalInput")
    if causal:
        pdk_d = nc.dram_tensor("paddk", [128, NKT], f32, kind="ExternalInput")
    else:
        msk_d = nc.dram_tensor("maskt", [L, L], f32, kind="ExternalInput")
    out_d = nc.dram_tensor("out", [L, D], f32, kind="ExternalOutput")

    with tile.TileContext(nc) as tc, (
        __import__("contextlib").ExitStack()) as ctx:
        ep = ctx.enter_context
        wpool = ep(tc.tile_pool(name="persist", bufs=1))
        qkpool = ep(tc.tile_pool(name="qk", bufs=1))
        vpool = ep(tc.tile_pool(name="vaug", bufs=1))
        mmp = ep(tc.tile_pool(name="mm", bufs=2, space="PSUM"))
        mm2p = ep(tc.tile_pool(name="mm2", bufs=2, space="PSUM"))
        ctxp = ep(tc.tile_pool(name="ctxps", bufs=2, space="PSUM"))
        ppool = ep(tc.tile_pool(name="pexp", bufs=4))
        ctpool = ep(tc.tile_pool(name="ctxt", bufs=8))
        rzpool = ep(tc.tile_pool(name="rzc", bufs=2))
        rbpool = ep(tc.tile_pool(name="rbc", bufs=2))
        zdpool = ep(tc.tile_pool(name="zdram", bufs=4, space="DRAM"))
        opool = ep(tc.tile_pool(name="outsb", bufs=2))

        wq_sb = wk_sb = wv_sb = bq_sb = bk_sb = bv_sb = None

        qt_sb = [qkpool.tile([128, L], mdt, tag=f"qt{j}", name="qt_sb") for j in range(NJ)]
        kt_sb = [qkpool.tile([128, L], mdt, tag=f"kt{j}", name="kt_sb") for j in range(NJ)]
        vaug = [vpool.tile([128, HG * 65], mdt, tag=f"va{k}", name="vaug") for k in range(NKT)]
        ones8 = wpool.tile([128, HG, 1], f32, tag="ones8")
        nc.vector.memset(ones8, 1.0)
        for kt in range(NKT):
            v3 = vaug[kt].rearrange("p (h d) -> p h d", h=HG)
            nc.scalar.copy(out=v3[:, :, 64:65], in_=ones8)

        def load_w3(which):
            nonlocal wq_sb, wk_sb, wv_sb, bq_sb, bk_sb, bv_sb
            if which == "q":
                wq_bg = wpool.tile([128, ND, DG], mdt, tag="wqb", name="wq_bg")
                nc.sync.dma_start(
                    out=wq_bg,
                    in_=wq_d[:, :].rearrange("(i p) n -> p i n", p=128))
                wq_sb = [wq_bg[:, i, :] for i in range(ND)]
                bq_sb = wpool.tile([128, NJ], f32, tag="bq")
                nc.sync.dma_start(out=bq_sb, in_=bq_d[:, :])
            elif which == "k":
                wk_bg = wpool.tile([128, ND, DG], mdt, tag="wkb", name="wk_bg")
                nc.sync.dma_start(
                    out=wk_bg,
                    in_=wk_d[:, :].rearrange("(i p) n -> p i n", p=128))
                wk_sb = [wk_bg[:, i, :] for i in range(ND)]
                bk_sb = wpool.tile([128, NJ], f32, tag="bk")
                nc.sync.dma_start(out=bk_sb, in_=bk_d[:, :])
            else:
                wv_bg = wpool.tile([128, ND, DG], mdt, tag="wvb", name="wv_bg")
                nc.sync.dma_start(
                    out=wv_bg,
                    in_=wv_d[:, :].rearrange("(i p) n -> p i n", p=128))
                wv_sb = [wv_bg[:, i, :] for i in range(ND)]
                bv_sb = wpool.tile([128, DG], f32, tag="bv")
                bv_ap = bv_d[:]
                bv_bcast = bass.AP(
                    tensor=bv_ap.tensor, offset=bv_ap.offset,
                    ap=[[0, 128]] + list(bv_ap.ap))
                nc.sync.dma_start(out=bv_sb, in_=bv_bcast)

        wo_sb = pdk_sb = None

        def setup_wo_masks():
            nonlocal wo_sb, pdk_sb
            wo_bg = wpool.tile([128, NJ, D], mdt, tag="wob", name="wo_bg")
            nc.sync.dma_start(
                out=wo_bg, in_=wo_d[:, :].rearrange("(j p) n -> p j n", p=128))
            wo_sb = [wo_bg[:, j, :] for j in range(NJ)]
            if causal:
                pdk_sb = wpool.tile([128, NKT], f32, tag="pdk")
                nc.sync.dma_start(out=pdk_sb, in_=pdk_d[:, :])

        def load_x(xd, t, xpool):
            ts = slice(512 * t, 512 * (t + 1))
            xt_bg = xpool.tile([128, ND, 512], mdt, tag="xt",
                               name="xt_bg", bufs=6)
            nc.sync.dma_start(
                out=xt_bg, in_=xd[:, ts].rearrange("(i p) n -> p i n", p=128))
            return [xt_bg[:, i, :] for i in range(ND)]

        def qk_chain(w_sb, b_sb, dest, j, xts, t):
            ts = slice(512 * t, 512 * (t + 1))
            ps = mmp.tile([128, 512], f32, tag="mm")
            for i in range(ND):
                nc.tensor.matmul(
                    out=ps, lhsT=w_sb[i][:, 128 * j:128 * (j + 1)],
                    rhs=xts[i], start=(i == 0), stop=(i == ND - 1))
            nc.vector.tensor_scalar_add(
                out=dest[j][:, ts], in0=ps, scalar1=b_sb[:, j:j + 1])

        def v_chain(xts, s, t):
            ps = mmp.tile([128, 512], f32, tag="mm")
            for i in range(ND):
                nc.tensor.matmul(
                    out=ps, lhsT=xts[i][:, 128 * s:128 * (s + 1)],
                    rhs=wv_sb[i], start=(i == 0), stop=(i == ND - 1))
            kt = 4 * t + s
            v3 = vaug[kt].rearrange("p (h d) -> p h d", h=HG)
            nc.vector.tensor_add(
                v3[:, :, 0:64],
                ps.rearrange("p (h d) -> p h d", h=HG),
                bv_sb.rearrange("p (h d) -> p h d", h=HG))

        def proj_fillers(t, xpool):
            """Closures, one matmul chain each, for proj(t); x DMAs eager."""
            xq = load_x(xqt, t, xpool)
            xk = load_x(xkt, t, xpool)
            xv = load_x(xvt, t, xpool)
            fs = []
            for j in range(NJ):
                fs.append(lambda j=j: qk_chain(wq_sb, bq_sb, qt_sb, j, xq, t))
            for j in range(NJ):
                fs.append(lambda j=j: qk_chain(wk_sb, bk_sb, kt_sb, j, xk, t))
            for s in range(4):
                fs.append(lambda s=s: v_chain(xv, s, t))
            return fs

        def out_proj(t):
            for s in range(4):
                for e in range(2):
                    es = slice(512 * e, 512 * (e + 1))
                    ps = mmp.tile([128, 512], f32, tag="mm")
                    for jt in range(NJ):
                        nc.tensor.matmul(
                            out=ps,
                            lhsT=ctxt_cur[jt][:, 128 * s:128 * (s + 1)],
                            rhs=wo_sb[jt][:, es],
                            start=(jt == 0), stop=(jt == NJ - 1))
                    ob = opool.tile([128, 512], f32, tag="ob")
                    nc.vector.tensor_copy(out=ob, in_=ps)
                    r0 = 512 * t + 128 * s
                    nc.sync.dma_start(out=out_d[r0:r0 + 128, es], in_=ob)

        ctxt_cur = None

        def emit_attn(t, fillers=(), mpool=None):
            nonlocal ctxt_cur
            qs = slice(512 * t, 512 * (t + 1))
            nkt_t = 4 * t + 4 if causal else NKT
            if not causal:
                msk = []
                for hkt in range(4):
                    msk_bg = mpool.tile([128, NKT // 4, 512], f32, tag="msk",
                                        name="msk_bg", bufs=6)
                    rs = slice(512 * hkt, 512 * (hkt + 1))
                    nc.sync.dma_start(
                        out=msk_bg,
                        in_=msk_d[rs, qs].rearrange("(k p) n -> p k n", p=128))
                    msk.extend(msk_bg[:, kt, :] for kt in range(NKT // 4))
            fill = list(fillers)
            fi = 0
            ctxt = [ctpool.tile([128, 512], mdt, tag="ct", name="ctxt") for _ in range(NJ)]
            ctxt_cur = ctxt
            for hp in range(NJ):
                jt = hp
                ctx_ab = [ctxp.tile([128, 512], f32, tag="ctx", name="ctx_ab") for _ in range(2)]
                for kt in range(nkt_t):
                    ks = slice(128 * kt, 128 * (kt + 1))
                    # both heads of the pair share one 2-bank PSUM tile so a
                    # single exp (and a single diag mask) covers them
                    ps2 = mm2p.tile([128, 1024], f32, tag="mm2")
                    for half in range(2):
                        ro = 64 * half
                        nc.tensor.matmul(
                            out=ps2[:, 512 * half:512 * (half + 1)],
                            lhsT=kt_sb[jt][ro:ro + 64, ks],
                            rhs=qt_sb[jt][ro:ro + 64, qs],
                            start=True, stop=True, skip_group_check=True)
                    pe = ppool.tile([128, 1024], mdt, tag="pexp")
                    if causal:
                        bias = pdk_sb[:, kt:kt + 1]
                    else:
                        for half in range(2):
                            nc.vector.tensor_add(
                                ps2[:, 512 * half:512 * (half + 1)],
                                ps2[:, 512 * half:512 * (half + 1)], msk[kt])
                        bias = 0.0
                    nc.scalar.activation(out=pe, in_=ps2, func=AF.Exp,
                                         bias=bias)
                    if causal and kt >= 4 * t:
                        # zero q < k of this diag tile, both heads at once:
                        # keep where q - k - 128r >= 0
                        r = kt - 4 * t
                        w = 128 * (r + 1)
                        pe3 = pe.rearrange("p (h n) -> p h n", h=2)
                        nc.gpsimd.affine_select(
                            out=pe3[:, :, 0:w], in_=pe3[:, :, 0:w],
                            pattern=[[0, 2], [1, w]],
                            compare_op=ALU.is_ge, fill=0.0,
                            base=-128 * r, channel_multiplier=-1)
                    for half in range(2):
                        h = 2 * hp + half
                        nc.tensor.matmul(
                            out=ctx_ab[half][0:65, :],
                            lhsT=vaug[kt][:, 65 * h:65 * (h + 1)],
                            rhs=pe[:, 512 * half:512 * (half + 1)],
                            start=(kt == 0), stop=(kt == nkt_t - 1))
                # fill PE with next block's projection chains while DVE/ACT
                # drain this head-pair's normalization
                for _ in range(3):
                    if fi < len(fill):
                        fill[fi]()
                        fi += 1
                for half in range(2):
                    ro = 64 * half
                    cab = ctx_ab[half]
                    # row 64 is the softmax denominator Z: recip it straight
                    # out of PSUM, broadcast 1/Z to 64 partitions via a DRAM
                    # round trip (partition-stride-0 reads need a DRAM src),
                    # then normalize while copying PSUM->SBUF (fused on DVE
                    # scalar_tensor_tensor; only in0 may live in PSUM)
                    rz = rzpool.tile([1, 512], f32, tag="rz")
                    with nc.allow_low_precision(reason="1/Z bcast operand"):
                        nc.vector.reciprocal(out=rz, in_=cab[64:65, :])
                    zd = zdpool.tile([1, 512], f32, tag="zd", name="zd")
                    nc.sync.dma_start(out=zd, in_=rz)
                    zrow = zd[0, :]
                    rb_src = bass.AP(
                        tensor=zrow.tensor, offset=zrow.offset,
                        ap=[[0, 64]] + list(zrow.ap))
                    rb = rbpool.tile([64, 512], f32, tag="rb")
                    nc.sync.dma_start(out=rb, in_=rb_src)
                    nc.vector.scalar_tensor_tensor(
                        out=ctxt[jt][ro:ro + 64, :],
                        in0=cab[0:64, :],
                        scalar=1.0, in1=rb,
                        op0=ALU.mult, op1=ALU.mult)
            while fi < len(fill):
                fill[fi]()
                fi += 1

        if causal:
            xpool = ep(tc.tile_pool(name="xin", bufs=1))
            # startup: get the first Q chain running as early as possible;
            # each projection's weights are loaded right before its x tile
            load_w3("q")
            xq0 = load_x(xqt, 0, xpool)
            for j in range(NJ):
                qk_chain(wq_sb, bq_sb, qt_sb, j, xq0, 0)
            load_w3("k")
            xk0 = load_x(xkt, 0, xpool)
            for j in range(NJ):
                qk_chain(wk_sb, bk_sb, kt_sb, j, xk0, 0)
            load_w3("v")
            xv0 = load_x(xvt, 0, xpool)
            for s in range(4):
                v_chain(xv0, s, 0)
            setup_wo_masks()
            for t in range(NT):
                fillers = proj_fillers(t + 1, xpool) if t + 1 < NT else ()
                emit_attn(t, fillers)
                out_proj(t)
        else:
            with tc.tile_pool(name="xin", bufs=1) as xpool:
                load_w3("q")
                load_w3("k")
                load_w3("v")
                setup_wo_masks()
                for t in range(NT):
                    for f in proj_fillers(t, xpool):
                        f()
            mpool = ep(tc.tile_pool(name="msk", bufs=1))
            for t in range(NT):
                emit_attn(t, (), mpool)
                out_proj(t)

    nc.finalize()
    return nc


def _get_nc(causal):
    key = (MM_DTYPE, causal)
    if key not in _cache:
        _cache[key] = _build(MM_DTYPE, causal)
    return _cache[key]


last_result = None


def _is_causal(attn_mask):
    tri = np.tril(np.ones((L, L), bool))
    expect = np.where(tri, np.float32(0.0), np.float32(-1e9))
    return np.array_equal(attn_mask, expect)


def kernel(**inputs):
    global last_result
    from concourse.bass_utils import run_bass_kernel_spmd

    inp = {k: np.asarray(v) for k, v in inputs.items()}
    scale = 1.0 / np.sqrt(np.float32(DH))
    wq_s = (inp["Wq"].astype(np.float32) * scale).astype(bfloat16)
    bq_s = (inp["bq"].astype(np.float32) * scale).astype(np.float32)
    padd = inp["padd_mask"].astype(np.float32)
    am = inp["attn_mask"].astype(np.float32)
    causal = _is_causal(am)

    if not causal:
        maskT = np.ascontiguousarray(am.T)

    in_maps = []
    for b in range(B):
        xq = inp["encodings_for_q"][b].astype(np.float32).T.astype(bfloat16)
        xk = inp["encodings_for_k"][b].astype(np.float32).T.astype(bfloat16)
        xv = inp["encodings_for_v"][b].astype(np.float32).T.astype(bfloat16)
        if causal:
            mask_entries = {
                "paddk": np.ascontiguousarray(padd[b].reshape(NKT, 128).T),
            }
        else:
            mask_entries = {
                "maskt": (maskT + padd[b][:, None]).astype(np.float32)}
        for g in range(G):
            gs = slice(DG * g, DG * (g + 1))
            in_maps.append({
                "xqt": xq, "xkt": xk, "xvt": xv,
                "wq": np.ascontiguousarray(wq_s[:, gs]),
                "wk": np.ascontiguousarray(
                    inp["Wk"].astype(np.float32)[:, gs].astype(bfloat16)),
                "wv": np.ascontiguousarray(
                    inp["Wv"].astype(np.float32)[:, gs].astype(bfloat16)),
                "bq": np.ascontiguousarray(bq_s[gs].reshape(NJ, 128).T),
                "bk": np.ascontiguousarray(
                    inp["bk"].astype(np.float32)[gs].reshape(NJ, 128).T),
                "bv": np.ascontiguousarray(inp["bv"].astype(np.float32)[gs]),
                "wo": np.ascontiguousarray(
                    inp["Wo"].astype(np.float32)[gs, :].astype(bfloat16)),
                **mask_entries,
            })

    import os
    trace = bool(os.environ.get("KBENCH_TRACE"))
    try:
        nc = _get_nc(causal)
        res = run_bass_kernel_spmd(nc, in_maps, list(range(NCORES)), trace=trace)
    except Exception:
        if not causal:
            raise
        # causal fast-path NEFF failed at runtime: fall back to the
        # sequential generic-mask variant (mask supplied as data)
        maskT_fb = np.ascontiguousarray(am.T)
        for b in range(B):
            mt = (maskT_fb + padd[b][:, None]).astype(np.float32)
            for g in range(G):
                m = in_maps[2 * b + g]
                m.pop("paddk", None)
                m["maskt"] = mt
        nc = _get_nc(False)
        res = run_bass_kernel_spmd(nc, in_maps, list(range(NCORES)), trace=trace)
    last_result = res
    out = np.empty((B, L, D), np.float32)
    for b in range(B):
        out[b] = res.results[2 * b]["out"] + res.results[2 * b + 1]["out"]
    return out


# revision 33
# speedup vs baseline: 1.5763x; 1.5763x over previous
"""Multi-head attention (B=4, L=2048, D=1024, H=16) on 8 trn2 NeuronCores.

Sharding: core c = 2*b + g handles batch b and head-group g (8 heads = 512 dims).
Each core computes Q/K/V projections for its group, attention for its 8 heads,
and a partial output projection ctx_g @ Wo[g*512:(g+1)*512, :].  The host sums
the two group partials per batch.

v6 (per core):
  storage dtype bf16 end-to-end (x, weights, QT/KT/V, probs, ctx); PSUM fp32
  QT, KT   : (512, 2048) feature-major (4 tiles of (128, L), 2 heads per tile)
  V        : (2048, 520) token-major, 65 cols per head (64 V dims + ones col
             accumulating the softmax denominator during the ctx matmul)
  scores   : head-pair fused in one (128 k, 1024 q) 2-bank PSUM tile;
             ONE exp per key tile (padd folded into the exp bias); causal
             diag tiles compute only the live query range [128r, 512) and
             the partially-masked 128-wide strip is zeroed AFTER exp by one
             gpsimd affine_select (no mask tensors, no DVE adds)
  norm     : Z row recip on DVE -> 1/Z broadcast to 64 partitions by gpsimd
             partition_broadcast (attn ucode library) -> fused (ctx*1)*(1/Z)
             PSUM->SBUF copy-out on DVE scalar_tensor_tensor
  schedule : engines run in emission order, so proj(t+1) Q/K chains, V(t)
             chains, and out-proj(t-1) chains are EMITTED interleaved into
             attn(t)'s head/key loops to keep PE busy through the ACT(exp)-
             heavy attention phases; startup w/x DMAs split+interleaved in
             halves; the final block's out-proj borrows the idle score pool
  out      : ctxT.T @ Wo chunks; fp32 partials DMAd out, host sums pairs
"""

import sys

if "/opt/trn_rl_repo" not in sys.path:
    sys.path.insert(0, "/opt/trn_rl_repo")

import numpy as np
from ml_dtypes import bfloat16

B, L, D, H = 4, 2048, 1024, 16
G = 2                # head-groups == cores per batch
DG = D // G          # 512 dims per group
HG = H // G          # 8 heads per group
DH = D // H          # 64
NCORES = B * G
NT = L // 512        # query 512-blocks
NKT = L // 128       # key 128-tiles
ND = D // 128        # contraction chunks over input dim
NJ = DG // 128       # dcol tiles per group (2 heads each)

MM_DTYPE = "bfloat16"

_cache = {}


def _build(mm_dtype_name, causal):
    import concourse.bass as bass
    import concourse.tile as tile
    from concourse import bacc, library_config, mybir

    f32 = mybir.dt.float32
    AF = mybir.ActivationFunctionType
    ALU = mybir.AluOpType
    mdt = getattr(mybir.dt, mm_dtype_name)

    nc = bacc.Bacc("TRN2")

    xqt = nc.dram_tensor("xqt", [D, L], mdt, kind="ExternalInput")
    xkt = nc.dram_tensor("xkt", [D, L], mdt, kind="ExternalInput")
    xvt = nc.dram_tensor("xvt", [D, L], mdt, kind="ExternalInput")
    wq_d = nc.dram_tensor("wq", [D, DG], mdt, kind="ExternalInput")
    wk_d = nc.dram_tensor("wk", [D, DG], mdt, kind="ExternalInput")
    wv_d = nc.dram_tensor("wv", [D, DG], mdt, kind="ExternalInput")
    bq_d = nc.dram_tensor("bq", [128, NJ], f32, kind="ExternalInput")
    bk_d = nc.dram_tensor("bk", [128, NJ], f32, kind="ExternalInput")
    bv_d = nc.dram_tensor("bv", [DG], f32, kind="ExternalInput")
    wo_d = nc.dram_tensor("wo", [DG, D], mdt, kind="ExternalInput")
    if causal:
        pdk_d = nc.dram_tensor("paddk", [128, NKT], f32, kind="ExternalInput")
    else:
        msk_d = nc.dram_tensor("maskt", [L, L], f32, kind="ExternalInput")
    out_d = nc.dram_tensor("out", [L, D], f32, kind="ExternalOutput")

    with tile.TileContext(nc) as tc, (
        __import__("contextlib").ExitStack()) as ctx:
        ep = ctx.enter_context
        wpool = ep(tc.tile_pool(name="persist", bufs=1))
        qkpool = ep(tc.tile_pool(name="qk", bufs=1))
        vpool = ep(tc.tile_pool(name="vaug", bufs=1))
        mmp = ep(tc.tile_pool(name="mm", bufs=2, space="PSUM"))
        mm2p = ep(tc.tile_pool(name="mm2", bufs=2, space="PSUM"))
        ctxp = ep(tc.tile_pool(name="ctxps", bufs=2, space="PSUM"))
        ppool = ep(tc.tile_pool(name="pexp", bufs=4))
        ctpool = ep(tc.tile_pool(name="ctxt", bufs=8))
        rzpool = ep(tc.tile_pool(name="rzc", bufs=2))
        rbpool = ep(tc.tile_pool(name="rbc", bufs=2))
        opool = ep(tc.tile_pool(name="outsb", bufs=4))

        # partition_broadcast (1/Z) needs the gpsimd attn library
        nc.gpsimd.load_library(library_config.attn)

        wq_sb = wk_sb = wv_sb = bq_sb = bk_sb = bv_sb = None

        qt_sb = [qkpool.tile([128, L], mdt, tag=f"qt{j}", name="qt_sb") for j in range(NJ)]
        kt_sb = [qkpool.tile([128, L], mdt, tag=f"kt{j}", name="kt_sb") for j in range(NJ)]
        vaug = [vpool.tile([128, HG * 65], mdt, tag=f"va{k}", name="vaug") for k in range(NKT)]
        ones8 = wpool.tile([128, HG, 1], f32, tag="ones8")
        nc.vector.memset(ones8, 1.0)
        for kt in range(NKT):
            v3 = vaug[kt].rearrange("p (h d) -> p h d", h=HG)
            nc.scalar.copy(out=v3[:, :, 64:65], in_=ones8)

        def load_w3(which):
            nonlocal wq_sb, wk_sb, wv_sb, bq_sb, bk_sb, bv_sb
            if which == "q":
                # split halves so the first Q-chain matmuls can start after
                # only half the weight load has landed (per-region deps)
                wq_bg = wpool.tile([128, ND, DG], mdt, tag="wqb", name="wq_bg")
                for hh in range(2):
                    nc.sync.dma_start(
                        out=wq_bg[:, 4 * hh:4 * (hh + 1), :],
                        in_=wq_d[512 * hh:512 * (hh + 1), :].rearrange(
                            "(i p) n -> p i n", p=128))
                wq_sb = [wq_bg[:, i, :] for i in range(ND)]
                bq_sb = wpool.tile([128, NJ], f32, tag="bq")
                nc.sync.dma_start(out=bq_sb, in_=bq_d[:, :])
            elif which == "k":
                wk_bg = wpool.tile([128, ND, DG], mdt, tag="wkb", name="wk_bg")
                nc.sync.dma_start(
                    out=wk_bg,
                    in_=wk_d[:, :].rearrange("(i p) n -> p i n", p=128))
                wk_sb = [wk_bg[:, i, :] for i in range(ND)]
                bk_sb = wpool.tile([128, NJ], f32, tag="bk")
                nc.sync.dma_start(out=bk_sb, in_=bk_d[:, :])
            else:
                wv_bg = wpool.tile([128, ND, DG], mdt, tag="wvb", name="wv_bg")
                nc.sync.dma_start(
                    out=wv_bg,
                    in_=wv_d[:, :].rearrange("(i p) n -> p i n", p=128))
                wv_sb = [wv_bg[:, i, :] for i in range(ND)]
                bv_sb = wpool.tile([128, DG], f32, tag="bv")
                bv_ap = bv_d[:]
                bv_bcast = bass.AP(
                    tensor=bv_ap.tensor, offset=bv_ap.offset,
                    ap=[[0, 128]] + list(bv_ap.ap))
                nc.sync.dma_start(out=bv_sb, in_=bv_bcast)

        wo_sb = pdk_sb = None

        def setup_wo_masks():
            nonlocal wo_sb, pdk_sb
            wo_bg = wpool.tile([128, NJ, D], mdt, tag="wob", name="wo_bg")
            nc.sync.dma_start(
                out=wo_bg, in_=wo_d[:, :].rearrange("(j p) n -> p j n", p=128))
            wo_sb = [wo_bg[:, j, :] for j in range(NJ)]
            if causal:
                pdk_sb = wpool.tile([128, NKT], f32, tag="pdk")
                nc.sync.dma_start(out=pdk_sb, in_=pdk_d[:, :])

        def load_x(xd, t, xpool, n_split=1):
            ts = slice(512 * t, 512 * (t + 1))
            xt_bg = xpool.tile([128, ND, 512], mdt, tag="xt",
                               name="xt_bg", bufs=6)
            step = ND // n_split
            for hh in range(n_split):
                nc.sync.dma_start(
                    out=xt_bg[:, step * hh:step * (hh + 1), :],
                    in_=xd[128 * step * hh:128 * step * (hh + 1),
                           ts].rearrange("(i p) n -> p i n", p=128))
            return [xt_bg[:, i, :] for i in range(ND)]

        def qk_chain(w_sb, b_sb, dest, j, xts, t):
            ts = slice(512 * t, 512 * (t + 1))
            ps = mmp.tile([128, 512], f32, tag="mm")
            for i in range(ND):
                nc.tensor.matmul(
                    out=ps, lhsT=w_sb[i][:, 128 * j:128 * (j + 1)],
                    rhs=xts[i], start=(i == 0), stop=(i == ND - 1))
            nc.vector.tensor_scalar_add(
                out=dest[j][:, ts], in0=ps, scalar1=b_sb[:, j:j + 1])

        def v_chain(xts, s, t):
            ps = mmp.tile([128, 512], f32, tag="mm")
            for i in range(ND):
                nc.tensor.matmul(
                    out=ps, lhsT=xts[i][:, 128 * s:128 * (s + 1)],
                    rhs=wv_sb[i], start=(i == 0), stop=(i == ND - 1))
            kt = 4 * t + s
            v3 = vaug[kt].rearrange("p (h d) -> p h d", h=HG)
            nc.vector.tensor_add(
                v3[:, :, 0:64],
                ps.rearrange("p (h d) -> p h d", h=HG),
                bv_sb.rearrange("p (h d) -> p h d", h=HG))

        def qkv_fillers(t, xpool):
            """Chain closures for proj(t), x DMAs issued eagerly now.

            Returns (qk_chains, v_chains): Q/K chains must be emitted before
            attn(t); V chains only before attn(t)'s kt reaches 4t (they are
            fed to attn(t) as early in-loop fillers to stretch PE work into
            the late, exp-heavy attention blocks)."""
            xq = load_x(xqt, t, xpool)
            xk = load_x(xkt, t, xpool)
            xv = load_x(xvt, t, xpool)
            qk = []
            for j in range(NJ):
                qk.append(lambda j=j: qk_chain(wq_sb, bq_sb, qt_sb, j, xq, t))
            for j in range(NJ):
                qk.append(lambda j=j: qk_chain(wk_sb, bk_sb, kt_sb, j, xk, t))
            vs = [lambda s=s: v_chain(xv, s, t) for s in range(4)]
            return qk, vs

        def out_proj_closures(t, ctxt, use_mm2=False):
            def one(s, e):
                def run():
                    es = slice(512 * e, 512 * (e + 1))
                    # the final block's chains may borrow the (then idle)
                    # score pool for 4-deep PSUM pipelining in the drain
                    if use_mm2 and (s + 2 * e) % 2 == 1:
                        ps2 = mm2p.tile([128, 1024], f32, tag="mm2",
                                        name="ps2")
                        ps = ps2[:, 0:512]
                    else:
                        ps = mmp.tile([128, 512], f32, tag="mm")
                    for jt in range(NJ):
                        nc.tensor.matmul(
                            out=ps,
                            lhsT=ctxt[jt][:, 128 * s:128 * (s + 1)],
                            rhs=wo_sb[jt][:, es],
                            start=(jt == 0), stop=(jt == NJ - 1))
                    ob = opool.tile([128, 512], f32, tag="ob")
                    nc.vector.tensor_copy(out=ob, in_=ps)
                    r0 = 512 * t + 128 * s
                    nc.sync.dma_start(out=out_d[r0:r0 + 128, es], in_=ob)
                return run
            return [one(s, e) for s in range(4) for e in range(2)]

        ctxt_cur = None

        def emit_attn(t, fillers=(), early=(), mpool=None):
            nonlocal ctxt_cur
            qs = slice(512 * t, 512 * (t + 1))
            nkt_t = 4 * t + 4 if causal else NKT
            early = list(early)
            if not causal:
                msk = []
                for hkt in range(4):
                    msk_bg = mpool.tile([128, NKT // 4, 512], f32, tag="msk",
                                        name="msk_bg", bufs=6)
                    rs = slice(512 * hkt, 512 * (hkt + 1))
                    nc.sync.dma_start(
                        out=msk_bg,
                        in_=msk_d[rs, qs].rearrange("(k p) n -> p k n", p=128))
                    msk.extend(msk_bg[:, kt, :] for kt in range(NKT // 4))
            fill = list(fillers)
            fi = 0
            nslot = (len(fill) + NJ - 1) // NJ if fill else 0
            ctxt = [ctpool.tile([128, 512], mdt, tag="ct", name="ctxt") for _ in range(NJ)]
            ctxt_cur = ctxt
            for hp in range(NJ):
                jt = hp
                ctx_ab = [ctxp.tile([128, 512], f32, tag="ctx", name="ctx_ab") for _ in range(2)]
                for kt in range(nkt_t):
                    # spread this block's own V chains through the non-diag
                    # part of the first head-pair's key loop (each vaug[4t+s]
                    # is first read at kt == 4t+s)
                    if early and (kt % 2 == 1 or kt >= 4 * t - 1):
                        early.pop(0)()
                    ks = slice(128 * kt, 128 * (kt + 1))
                    # diag tiles (kt >= 4t): queries below 128r are fully
                    # masked -- compute only the valid q sub-range [q0:512)
                    r = kt - 4 * t if causal else -1
                    q0 = 128 * r if r > 0 else 0
                    # both heads of the pair share one 2-bank PSUM tile so a
                    # single exp (and a single diag mask) covers them
                    ps2 = mm2p.tile([128, 1024], f32, tag="mm2")
                    ps3 = ps2.rearrange("p (h n) -> p h n", h=2)
                    for half in range(2):
                        ro = 64 * half
                        nc.tensor.matmul(
                            out=ps3[:, half, q0:512],
                            lhsT=kt_sb[jt][ro:ro + 64, ks],
                            rhs=qt_sb[jt][ro:ro + 64,
                                          512 * t + q0:512 * (t + 1)],
                            start=True, stop=True, skip_group_check=True)
                    pe = ppool.tile([128, 1024], mdt, tag="pexp")
                    pe3 = pe.rearrange("p (h n) -> p h n", h=2)
                    if causal:
                        bias = pdk_sb[:, kt:kt + 1]
                    else:
                        for half in range(2):
                            nc.vector.tensor_add(
                                ps3[:, half, :], ps3[:, half, :], msk[kt])
                        bias = 0.0
                    nc.scalar.activation(
                        out=pe3[:, :, q0:512], in_=ps3[:, :, q0:512],
                        func=AF.Exp, bias=bias)
                    if causal and r >= 0:
                        # zero q < k within the 128-wide partially-masked
                        # strip [q0:q0+128), both heads at once: with local
                        # j = q - 128r the condition is keep iff j - k >= 0
                        nc.gpsimd.affine_select(
                            out=pe3[:, :, q0:q0 + 128],
                            in_=pe3[:, :, q0:q0 + 128],
                            pattern=[[0, 2], [1, 128]],
                            compare_op=ALU.is_ge, fill=0.0,
                            base=0, channel_multiplier=-1)
                    for half in range(2):
                        h = 2 * hp + half
                        nc.tensor.matmul(
                            out=ctx_ab[half][0:65, q0:512],
                            lhsT=vaug[kt][:, 65 * h:65 * (h + 1)],
                            rhs=pe3[:, half, q0:512],
                            start=(kt == 0), stop=(kt == nkt_t - 1),
                            skip_group_check=True)
                # fill PE with ready work (prev block's output projection,
                # next block's input projections) while DVE/ACT drain this
                # head-pair's normalization
                for _ in range(nslot):
                    if fi < len(fill):
                        fill[fi]()
                        fi += 1
                for half in range(2):
                    ro = 64 * half
                    cab = ctx_ab[half]
                    # row 64 is the softmax denominator Z: recip it straight
                    # out of PSUM, broadcast 1/Z to 64 partitions on the idle
                    # gpsimd engine, then normalize while copying PSUM->SBUF
                    # (fused on DVE scalar_tensor_tensor; only in0 in PSUM)
                    rz = rzpool.tile([1, 512], f32, tag="rz")
                    with nc.allow_low_precision(reason="1/Z bcast operand"):
                        nc.vector.reciprocal(out=rz, in_=cab[64:65, :])
                    rb = rbpool.tile([64, 512], f32, tag="rb")
                    nc.gpsimd.partition_broadcast(
                        rb[:, :], rz[:, :], channels=64)
                    nc.vector.scalar_tensor_tensor(
                        out=ctxt[jt][ro:ro + 64, :],
                        in0=cab[0:64, :],
                        scalar=1.0, in1=rb,
                        op0=ALU.mult, op1=ALU.mult)
            while fi < len(fill):
                fill[fi]()
                fi += 1

        if causal:
            xpool = ep(tc.tile_pool(name="xin", bufs=1))
            # startup: get the first Q chain running as early as possible --
            # wq/xq DMAs split in halves and interleaved so the chain's first
            # matmuls start after only the first halves have landed
            wq_bg = wpool.tile([128, ND, DG], mdt, tag="wqb", name="wq_bg")
            xq_bg = xpool.tile([128, ND, 512], mdt, tag="xt",
                               name="xt_bg", bufs=6)
            for hh in range(2):
                nc.sync.dma_start(
                    out=wq_bg[:, 4 * hh:4 * (hh + 1), :],
                    in_=wq_d[512 * hh:512 * (hh + 1), :].rearrange(
                        "(i p) n -> p i n", p=128))
                nc.sync.dma_start(
                    out=xq_bg[:, 4 * hh:4 * (hh + 1), :],
                    in_=xqt[512 * hh:512 * (hh + 1), 0:512].rearrange(
                        "(i p) n -> p i n", p=128))
            wq_sb = [wq_bg[:, i, :] for i in range(ND)]
            bq_sb = wpool.tile([128, NJ], f32, tag="bq")
            nc.sync.dma_start(out=bq_sb, in_=bq_d[:, :])
            xq0 = [xq_bg[:, i, :] for i in range(ND)]
            for j in range(NJ):
                qk_chain(wq_sb, bq_sb, qt_sb, j, xq0, 0)
            # same interleaved half-loading for K and V
            wk_bg = wpool.tile([128, ND, DG], mdt, tag="wkb", name="wk_bg")
            xk_bg = xpool.tile([128, ND, 512], mdt, tag="xt",
                               name="xt_bg", bufs=6)
            wv_bg = wpool.tile([128, ND, DG], mdt, tag="wvb", name="wv_bg")
            xv_bg = xpool.tile([128, ND, 512], mdt, tag="xt",
                               name="xt_bg", bufs=6)
            for hh in range(2):
                nc.sync.dma_start(
                    out=wk_bg[:, 4 * hh:4 * (hh + 1), :],
                    in_=wk_d[512 * hh:512 * (hh + 1), :].rearrange(
                        "(i p) n -> p i n", p=128))
                nc.sync.dma_start(
                    out=xk_bg[:, 4 * hh:4 * (hh + 1), :],
                    in_=xkt[512 * hh:512 * (hh + 1), 0:512].rearrange(
                        "(i p) n -> p i n", p=128))
            wk_sb = [wk_bg[:, i, :] for i in range(ND)]
            bk_sb = wpool.tile([128, NJ], f32, tag="bk")
            nc.sync.dma_start(out=bk_sb, in_=bk_d[:, :])
            for hh in range(2):
                nc.sync.dma_start(
                    out=wv_bg[:, 4 * hh:4 * (hh + 1), :],
                    in_=wv_d[512 * hh:512 * (hh + 1), :].rearrange(
                        "(i p) n -> p i n", p=128))
                nc.sync.dma_start(
                    out=xv_bg[:, 4 * hh:4 * (hh + 1), :],
                    in_=xvt[512 * hh:512 * (hh + 1), 0:512].rearrange(
                        "(i p) n -> p i n", p=128))
            wv_sb = [wv_bg[:, i, :] for i in range(ND)]
            bv_sb = wpool.tile([128, DG], f32, tag="bv")
            bv_ap = bv_d[:]
            bv_bcast = bass.AP(
                tensor=bv_ap.tensor, offset=bv_ap.offset,
                ap=[[0, 128]] + list(bv_ap.ap))
            nc.sync.dma_start(out=bv_sb, in_=bv_bcast)
            xk0 = [xk_bg[:, i, :] for i in range(ND)]
            xv0 = [xv_bg[:, i, :] for i in range(ND)]
            for j in range(NJ):
                qk_chain(wk_sb, bk_sb, kt_sb, j, xk0, 0)
            for s in range(4):
                v_chain(xv0, s, 0)
            setup_wo_masks()
            prev_out = []
            pend_v = []
            for t in range(NT):
                if t + 1 < NT:
                    qk_next, v_next = qkv_fillers(t + 1, xpool)
                else:
                    qk_next, v_next = [], []
                emit_attn(t, list(prev_out) + qk_next, early=pend_v)
                pend_v = v_next
                prev_out = out_proj_closures(t, ctxt_cur,
                                             use_mm2=(t == NT - 1))
            for f in prev_out:
                f()
        else:
            with tc.tile_pool(name="xin", bufs=1) as xpool:
                load_w3("q")
                load_w3("k")
                load_w3("v")
                setup_wo_masks()
                for t in range(NT):
                    qk, vs = qkv_fillers(t, xpool)
                    for f in qk + vs:
                        f()
            mpool = ep(tc.tile_pool(name="msk", bufs=1))
            for t in range(NT):
                emit_attn(t, (), (), mpool)
                for f in out_proj_closures(t, ctxt_cur):
                    f()

    nc.finalize()
    return nc


def _get_nc(causal):
    key = (MM_DTYPE, causal)
    if key not in _cache:
        _cache[key] = _build(MM_DTYPE, causal)
    return _cache[key]


last_result = None


def _is_causal(attn_mask):
    tri = np.tril(np.ones((L, L), bool))
    expect = np.where(tri, np.float32(0.0), np.float32(-1e9))
    return np.array_equal(attn_mask, expect)


def kernel(**inputs):
    global last_result
    from concourse.bass_utils import run_bass_kernel_spmd

    inp = {k: np.asarray(v) for k, v in inputs.items()}
    scale = 1.0 / np.sqrt(np.float32(DH))
    wq_s = (inp["Wq"].astype(np.float32) * scale).astype(bfloat16)
    bq_s = (inp["bq"].astype(np.float32) * scale).astype(np.float32)
    padd = inp["padd_mask"].astype(np.float32)
    am = inp["attn_mask"].astype(np.float32)
    causal = _is_causal(am)

    if not causal:
        maskT = np.ascontiguousarray(am.T)

    in_maps = []
    for b in range(B):
        xq = inp["encodings_for_q"][b].astype(np.float32).T.astype(bfloat16)
        xk = inp["encodings_for_k"][b].astype(np.float32).T.astype(bfloat16)
        xv = inp["encodings_for_v"][b].astype(np.float32).T.astype(bfloat16)
        if causal:
            mask_entries = {
                "paddk": np.ascontiguousarray(padd[b].reshape(NKT, 128).T),
            }
        else:
            mask_entries = {
                "maskt": (maskT + padd[b][:, None]).astype(np.float32)}
        for g in range(G):
            gs = slice(DG * g, DG * (g + 1))
            in_maps.append({
                "xqt": xq, "xkt": xk, "xvt": xv,
                "wq": np.ascontiguousarray(wq_s[:, gs]),
                "wk": np.ascontiguousarray(
                    inp["Wk"].astype(np.float32)[:, gs].astype(bfloat16)),
                "wv": np.ascontiguousarray(
                    inp["Wv"].astype(np.float32)[:, gs].astype(bfloat16)),
                "bq": np.ascontiguousarray(bq_s[gs].reshape(NJ, 128).T),
                "bk": np.ascontiguousarray(
                    inp["bk"].astype(np.float32)[gs].reshape(NJ, 128).T),
                "bv": np.ascontiguousarray(inp["bv"].astype(np.float32)[gs]),
                "wo": np.ascontiguousarray(
                    inp["Wo"].astype(np.float32)[gs, :].astype(bfloat16)),
                **mask_entries,
            })

    import os
    trace = bool(os.environ.get("KBENCH_TRACE"))
    try:
        nc = _get_nc(causal)
        res = run_bass_kernel_spmd(nc, in_maps, list(range(NCORES)), trace=trace)
    except Exception:
        if not causal:
            raise
        # causal fast-path NEFF failed at runtime: fall back to the
        # sequential generic-mask variant (mask supplied as data)
        maskT_fb = np.ascontiguousarray(am.T)
        for b in range(B):
            mt = (maskT_fb + padd[b][:, None]).astype(np.float32)
            for g in range(G):
                m = in_maps[2 * b + g]
                m.pop("paddk", None)
                m["maskt"] = mt
        nc = _get_nc(False)
        res = run_bass_kernel_spmd(nc, in_maps, list(range(NCORES)), trace=trace)
    last_result = res
    out = np.empty((B, L, D), np.float32)
    for b in range(B):
        out[b] = res.results[2 * b]["out"] + res.results[2 * b + 1]["out"]
    return out
